# revision 2
# baseline (speedup 1.0000x reference)
"""Self-contained Trainium2 Bass kernel for nn_Encoder_53369263620316.

kernel(**inputs) -> np.ndarray
  inputs (full, unsharded):
    ids        [256, 4096] int32/int64  token ids in [0, 50000]
    emb_table  [50001, 32] float32
    kernel     [32, 48]    float32   (Keras GRU v2 kernel, gate order z|r|h)
    rec_kernel [16, 48]    float32
    bias       [2, 48]     float32   (row 0 input bias, row 1 recurrent bias)
  returns h_final [256, 16] float32.

Sharding: data-parallel across 8 NeuronCores -- batch dim split 8 x 32;
embedding table (bf16) and GRU weights replicated.

Device algorithm per core (B=32 sequences, T=4096 steps):
  State h is kept decomposed as h = a + p2 in a per-step column of the
  activation buffer X (bf16):
    X rows: a 0:16 | junk 16:32 | p2 32:48 | junk 48:64 | embT 64:96 | ones 96
  Per step, per batch-group g (2 groups of 16, software-pipelined):
    PE : h-mm   H2p[0:16] = a + p2            (lhsT = [I;0;I])
         mm     P[0:112]  = S_w^T @ X[:,col]  (bf16; M layout
                z|pad|zn|pad|r|pad|rh, biases via ones row)
    ACT: sig3   sout[0:80] = sigmoid(P[0:80])   (z, zn=sig(-zpre), r)
         sighh  H2p[32:48] = sigmoid(u + b0h)   (bias AP)
    DVE: q = P[96:112] * sout[64:80]            (rh * r)
         blend  X[0:48, col+1] = sout[0:48] * H2p[0:48]  ([z;~;zn]*[h;~;hh])
    GPS: u = q + xh_sb                           (xh staged in SBUF)
  Embeddings are gathered by indirect DMA (128 tokens/call), PE-transposed
  into X rows 64:96; xh = K_h^T emb (+b0h later) is bulk-matmul'd 16 steps
  at a time into PSUM and DVE-copied to SBUF; all double-buffered across
  chunks of Tc steps, For_i body = 2 chunks.
"""

from contextlib import ExitStack

import numpy as np

import concourse.bass as bass
import concourse.bacc as bacc
import concourse.mybir as mybir
import concourse.tile as tile
from concourse.bass_utils import run_bass_kernel_spmd
from concourse.masks import make_identity

F32 = mybir.dt.float32
BF16 = mybir.dt.bfloat16
I32 = mybir.dt.int32
SIG = mybir.ActivationFunctionType.Sigmoid
ADD = mybir.AluOpType.add
MUL = mybir.AluOpType.mult
PEE = mybir.EngineType.PE
DVEE = mybir.EngineType.DVE
ACTE = mybir.EngineType.Activation

NCORES = 8
B = 32          # batch rows per core
G = 2           # pipelined batch groups
Bg = B // G
H = 16          # GRU units
E = 32          # embedding dim
T = 4096
TC = 256        # steps per chunk (2 chunks per For_i body)
VOCAB = 50001
XROWS = 97      # a|junk|p2|junk|embT|ones
UBLK = 16       # steps per xh block (512 cols)


def build_kernel(T, Tc, vocab=VOCAB, use_for_i=True):
    assert Tc % UBLK == 0 and T % (2 * Tc) == 0
    GPC = Tc * B // 128          # gather groups (128 tokens) per chunk
    NBLK = Tc // UBLK            # xh blocks (512 cols) per chunk
    NCHUNK = T // Tc
    NBODY = NCHUNK // 2
    n_groups = T * B // 128
    n_groups_pad = n_groups + 2 * GPC

    nc = bacc.Bacc(None, target_bir_lowering=False, debug=False)

    emb_d = nc.dram_tensor("emb_table", [vocab, E], BF16, kind="ExternalInput")
    sw_d = nc.dram_tensor("s_w", [XROWS, 112], BF16, kind="ExternalInput")
    wxh_d = nc.dram_tensor("w_xh", [E, H], BF16, kind="ExternalInput")
    b0h_d = nc.dram_tensor("b0h", [H, 1], F32, kind="ExternalInput")
    offs_d = nc.dram_tensor("offs", [128, n_groups_pad], I32, kind="ExternalInput")
    out_d = nc.dram_tensor("h_final", [H, B], F32, kind="ExternalOutput")

    with tile.TileContext(nc) as tc:
        with ExitStack() as ctx:
            sb = ctx.enter_context(tc.tile_pool(name="sb", bufs=1))
            ps = ctx.enter_context(tc.tile_pool(name="ps", bufs=1, space="PSUM"))

            S_w = sb.tile([XROWS, 112], BF16)
            wxh96 = sb.tile([96, H], BF16)      # rows 64:96 = K_h
            hI48 = sb.tile([48, H], BF16)       # [I;0;I]
            b0h = sb.tile([H, 1], F32)
            ident = sb.tile([128, 128], BF16)
            offs = sb.tile([128, n_groups_pad], I32)
            XA = sb.tile([XROWS, Tc * B], BF16)
            XB = sb.tile([XROWS, Tc * B], BF16)
            stgA = sb.tile([128, GPC * E], BF16)
            stgB = sb.tile([128, GPC * E], BF16)
            owinA = sb.tile([128, GPC], I32)
            owinB = sb.tile([128, GPC], I32)
            xh_sb = [sb.tile([H, 512], F32, name=f"xhsb_{i}") for i in range(2)]
            sout = [sb.tile([80, Bg], F32, name=f"sout_{g}") for g in range(G)]
            q_sb = [sb.tile([H, Bg], F32, name=f"q_{g}") for g in range(G)]
            u_sb = [sb.tile([H, Bg], F32, name=f"u_{g}") for g in range(G)]
            ofin = sb.tile([H, B], F32)

            # full-bank PSUM tiles (bank-exclusive by construction)
            P = [ps.tile([128, 512], F32, name=f"P_{g}") for g in range(G)]
            H2p = [ps.tile([128, 512], F32, name=f"H2p_{g}") for g in range(G)]
            XH = [ps.tile([128, 512], F32, name=f"XH_{i}") for i in range(2)]
            TP = [ps.tile([128, 1024], BF16, name=f"TP_{i}") for i in range(2)]

            nc.sync.dma_start(out=S_w[:], in_=sw_d[:])
            nc.sync.dma_start(out=wxh96[64:96, :], in_=wxh_d[:])
            nc.sync.dma_start(out=b0h[:], in_=b0h_d[:])
            nc.sync.dma_start(out=offs[:], in_=offs_d[:])
            make_identity(nc, ident[:])
            nc.vector.memset(hI48[:], 0.0)
            nc.vector.memset(wxh96[0:64, :], 0.0)
            nc.vector.memset(XA[:], 0.0)
            nc.vector.memset(XB[:], 0.0)
            nc.vector.memset(XA[96:97, :], 1.0)
            nc.vector.memset(XB[96:97, :], 1.0)
            for g in range(G):
                nc.vector.memset(sout[g][:], 0.0)
                nc.vector.memset(q_sb[g][:], 0.0)
                nc.vector.memset(u_sb[g][:], 0.0)
            for i in range(2):
                nc.vector.memset(xh_sb[i][:], 0.0)
            nc.vector.memset(ofin[:], 0.0)
            # hI48 = [I;0;I] via iota compare would be complex; use identity copy
            nc.vector.tensor_copy(hI48[0:16, :], ident[0:16, 0:16])
            nc.vector.tensor_copy(hI48[32:48, :], ident[0:16, 0:16])

            def emit_owin(chunk, owin):
                if isinstance(chunk, int):
                    src = offs[:, chunk * GPC:(chunk + 1) * GPC]
                else:
                    src = offs[:, bass.ts(chunk, GPC)]
                nc.vector.tensor_copy(owin[:], src)

            def emit_gather_one(stg, owin, g):
                nc.gpsimd.indirect_dma_start(
                    out=stg[:, g * E:(g + 1) * E],
                    out_offset=None,
                    in_=emb_d[:],
                    in_offset=bass.IndirectOffsetOnAxis(ap=owin[:, g:g + 1], axis=0),
                )

            def emit_gather(chunk, stg, owin):
                emit_owin(chunk, owin)
                for g in range(GPC):
                    emit_gather_one(stg, owin, g)

            def prep_ops(stg, X):
                """Transpose gathered embeddings into X rows 64:96."""
                for blk in range(NBLK):
                    def tp_blk(blk=blk):
                        tp = TP[blk % 2]
                        for j in range(4):
                            gg = blk * 4 + j
                            nc.tensor.transpose(
                                out=tp[0:E, j * 128:(j + 1) * 128],
                                in_=stg[:, gg * E:(gg + 1) * E],
                                identity=ident[:],
                            )
                        nc.vector.tensor_copy(
                            X[64:96, blk * 512:(blk + 1) * 512], tp[0:E, 0:512])
                    yield tp_blk

            def emit_xh(X, blk, xbuf):
                """xh for steps [blk*16,(blk+1)*16) of X -> XH[xbuf] -> xh_sb."""
                nc.tensor.matmul(
                    XH[xbuf][0:H, :], wxh96[64:96, :],
                    X[64:96, blk * 512:(blk + 1) * 512],
                    start=True, stop=True, tile_position=(64, 0))
                nc.vector.tensor_copy(xh_sb[xbuf][:], XH[xbuf][0:H, :])

            def emit_step(X, Xn, t, tn):
                """One timestep, both groups, stage-major."""
                cols = [t * B + g * Bg for g in range(G)]
                ncols = [tn * B + g * Bg for g in range(G)]
                us = (t % UBLK) * B
                xbuf = (t // UBLK) % 2
                for g in range(G):
                    nc.tensor.matmul(H2p[g][0:H, 0:Bg], hI48[:],
                                     X[0:48, cols[g]:cols[g] + Bg],
                                     start=True, stop=True)
                    nc.tensor.matmul(P[g][0:112, 0:Bg], S_w[:],
                                     X[0:XROWS, cols[g]:cols[g] + Bg],
                                     start=True, stop=True)
                for g in range(G):
                    nc.scalar.activation(sout[g][:], P[g][0:80, 0:Bg], SIG)
                for g in range(G):
                    nc.vector.tensor_tensor(q_sb[g][:], P[g][96:112, 0:Bg],
                                            sout[g][64:80, :], op=MUL)
                for g in range(G):
                    nc.gpsimd.tensor_tensor(
                        u_sb[g][:], q_sb[g][:],
                        xh_sb[xbuf][:, us + g * Bg:us + g * Bg + Bg], op=ADD)
                for g in range(G):
                    nc.scalar.activation(H2p[g][32:48, 0:Bg], u_sb[g][:], SIG,
                                         bias=b0h[:])
                for g in range(G):
                    nc.vector.tensor_tensor(Xn[0:48, ncols[g]:ncols[g] + Bg],
                                            sout[g][0:48, :],
                                            H2p[g][0:48, 0:Bg], op=MUL)

            def emit_chunk(X, Xn, next_chunk, stg, owin, preps):
                """Run Tc steps on X; interleave gather+prep of next chunk."""
                sched = {}
                # gather g of next chunk at step 4g (Tc/4 gathers per chunk);
                # transpose-block blk (needs gathers 4blk..4blk+3) at
                # step 16*blk + 14.
                sched.setdefault(0, []).append(lambda: emit_owin(next_chunk, owin))
                for g in range(GPC):
                    sched.setdefault((g * Tc) // GPC, []).append(
                        lambda g=g: emit_gather_one(stg, owin, g))
                for i, p in enumerate(preps):
                    sched.setdefault((i * Tc) // NBLK + 14, []).append(p)
                for t in range(Tc):
                    if t % UBLK == 0:
                        # xh for block t//UBLK + 1 (next block); last block
                        # preps block 0 of the NEXT chunk.
                        nb = t // UBLK + 1
                        if nb < NBLK:
                            emit_xh(X, nb, nb % 2)
                        else:
                            emit_xh(Xn, 0, 0)
                    tn = t + 1 if t + 1 < Tc else 0
                    emit_step(X, X if t + 1 < Tc else Xn, t, tn)
                    for p in sched.get(t, ()):
                        p()

            # --- prologue: gather+prep chunk 0 into A, xh block 0 ---
            emit_gather(0, stgA, owinA)
            for p in prep_ops(stgA, XA):
                p()
            emit_xh(XA, 0, 0)

            def body(i):
                emit_chunk(XA, XB, 2 * i + 1, stgB, owinB,
                           list(prep_ops(stgB, XB)))
                emit_chunk(XB, XA, 2 * i + 2, stgA, owinA,
                           list(prep_ops(stgA, XA)))

            if use_for_i:
                with tc.For_i(0, NBODY, 1,
                              hint_engines=(PEE, DVEE, ACTE)) as i:
                    body(i)
            else:
                for i in range(NBODY):
                    body(i)

            # final h = a + p2 from XA col 0 (state entering step T)
            nc.tensor.matmul(H2p[0][0:H, 0:B], hI48[:], XA[0:48, 0:B],
                             start=True, stop=True)
            nc.scalar.copy(out=ofin[:], in_=H2p[0][0:H, 0:B])
            nc.sync.dma_start(out=out_d[:], in_=ofin[:])

    nc.compile()
    return nc


def pack_inputs(ids_core, emb_table, kernel, rec_kernel, bias, T, Tc):
    """Host-side packing for one core. ids_core [32, T] int."""
    GPC = Tc * B // 128
    n_groups = T * B // 128
    n_groups_pad = n_groups + 2 * GPC
    R = np.asarray(rec_kernel, np.float32)          # [16, 48] cols z|r|h
    K = np.asarray(kernel, np.float32)              # [32, 48]
    b0, b1 = np.asarray(bias, np.float32)           # [48] each

    S = np.zeros((XROWS, 112), np.float32)
    # M cols: z 0:16 | pad | zn 32:48 | pad | r 64:80 | pad | rh 96:112
    for rows in (slice(0, 16), slice(32, 48)):      # a-rows, p2-rows
        S[rows, 0:16] = R[:, 0:16]
        S[rows, 32:48] = -R[:, 0:16]
        S[rows, 64:80] = R[:, 16:32]
        S[rows, 96:112] = R[:, 32:48]
    S[64:96, 0:16] = K[:, 0:16]
    S[64:96, 32:48] = -K[:, 0:16]
    S[64:96, 64:80] = K[:, 16:32]
    S[96, 0:16] = b0[0:16] + b1[0:16]
    S[96, 32:48] = -(b0[0:16] + b1[0:16])
    S[96, 64:80] = b0[16:32] + b1[16:32]
    S[96, 96:112] = b1[32:48]

    w_xh = np.asarray(K[:, 32:48], np.float32)      # [32, 16]
    b0h = b0[32:48].reshape(H, 1).astype(np.float32)

    flat = np.ascontiguousarray(ids_core.T).reshape(-1)   # i = t*32 + b
    offs = np.zeros((128, n_groups_pad), np.int32)
    offs[:, :n_groups] = flat.reshape(n_groups, 128).T.astype(np.int32)

    import ml_dtypes
    bf = lambda a: np.asarray(a, dtype=ml_dtypes.bfloat16)
    return {
        "emb_table": bf(emb_table),
        "s_w": bf(S),
        "w_xh": bf(w_xh),
        "b0h": b0h,
        "offs": offs,
    }


_NC_CACHE = {}


def _get_nc():
    key = (T, TC)
    if key not in _NC_CACHE:
        _NC_CACHE[key] = build_kernel(T=T, Tc=TC, vocab=VOCAB, use_for_i=True)
    return _NC_CACHE[key]


def make_in_maps(ids, emb_table, kern, rec_kernel, bias):
    ids = np.asarray(ids)
    assert ids.shape == (NCORES * B, T), ids.shape
    ids = ids.astype(np.int32, copy=False)
    return [
        pack_inputs(ids[c * B:(c + 1) * B], emb_table, kern, rec_kernel, bias,
                    T, TC)
        for c in range(NCORES)
    ]


def kernel(ids, emb_table, kernel, rec_kernel, bias):
    """Full inputs in, full output out. Shards batch 8 ways internally."""
    out_dtype = np.asarray(emb_table).dtype
    in_maps = make_in_maps(ids, emb_table, kernel, rec_kernel, bias)
    nc = _get_nc()
    res = run_bass_kernel_spmd(nc, in_maps, core_ids=list(range(NCORES)))
    out = np.concatenate(
        [res.results[c]["h_final"].T for c in range(NCORES)], axis=0
    ).astype(out_dtype, copy=False)
    return out


# revision 3
# speedup vs baseline: 1.0519x; 1.0519x over previous
"""Self-contained Trainium2 Bass kernel for nn_Encoder_53369263620316.

kernel(**inputs) -> np.ndarray
  inputs (full, unsharded):
    ids        [256, 4096] int32/int64  token ids in [0, 50000]
    emb_table  [50001, 32] float32
    kernel     [32, 48]    float32   (Keras GRU v2 kernel, gate order z|r|h)
    rec_kernel [16, 48]    float32
    bias       [2, 48]     float32   (row 0 input bias, row 1 recurrent bias)
  returns h_final [256, 16] float32.

Sharding: data-parallel across 8 NeuronCores -- batch dim split 8 x 32;
embedding table (bf16) and GRU weights replicated.

Device algorithm per core (B=32 sequences, T=4096 steps):
  State h is kept decomposed as h = a + p2 in a per-step column of the
  activation buffer X (bf16):
    X rows: a 0:16 | junk 16:32 | p2 32:48 | junk 48:64 | embT 64:96 | ones 96
  Per step, per batch-group g (2 groups of 16, software-pipelined):
    PE : h-mm   H2p[0:16] = a + p2            (lhsT = [I;0;I])
         mm     P[0:112]  = S_w^T @ X[:,col]  (bf16; M layout
                z|pad|zn|pad|r|pad|rh, biases via ones row)
    ACT: sig3   sout[0:80] = sigmoid(P[0:80])   (z, zn=sig(-zpre), r)
         sighh  H2p[32:48] = sigmoid(u + b0h)   (bias AP)
    DVE: q = P[96:112] * sout[64:80]            (rh * r)
         blend  X[0:48, col+1] = sout[0:48] * H2p[0:48]  ([z;~;zn]*[h;~;hh])
    GPS: u = q + xh_sb                           (xh staged in SBUF)
  Embeddings are gathered by indirect DMA (128 tokens/call), PE-transposed
  into X rows 64:96; xh = K_h^T emb (+b0h later) is bulk-matmul'd 16 steps
  at a time into PSUM and DVE-copied to SBUF; all double-buffered across
  chunks of Tc steps, For_i body = 2 chunks.
"""

from contextlib import ExitStack

import os

import numpy as np

import concourse.bass as bass
import concourse.bacc as bacc
import concourse.mybir as mybir
import concourse.tile as tile
from concourse.bass_utils import run_bass_kernel_spmd
from concourse.masks import make_identity

F32 = mybir.dt.float32
BF16 = mybir.dt.bfloat16
I32 = mybir.dt.int32
SIG = mybir.ActivationFunctionType.Sigmoid
ADD = mybir.AluOpType.add
MUL = mybir.AluOpType.mult
PEE = mybir.EngineType.PE
DVEE = mybir.EngineType.DVE
ACTE = mybir.EngineType.Activation

UENG = os.environ.get("KNOB_UENG", "gps")
NOGATHER = os.environ.get("KNOB_NOGATHER", "0") == "1"

NCORES = 8
B = 32          # batch rows per core
G = 2           # pipelined batch groups
Bg = B // G
H = 16          # GRU units
E = 32          # embedding dim
T = 4096
TC = 256        # steps per chunk (2 chunks per For_i body)
VOCAB = 50001
XROWS = 97      # a|junk|p2|junk|embT|ones
UBLK = 16       # steps per xh block (512 cols)


def build_kernel(T, Tc, vocab=VOCAB, use_for_i=True):
    assert Tc % UBLK == 0 and T % (2 * Tc) == 0
    GPC = Tc * B // 128          # gather groups (128 tokens) per chunk
    NBLK = Tc // UBLK            # xh blocks (512 cols) per chunk
    NCHUNK = T // Tc
    NBODY = NCHUNK // 2
    n_groups = T * B // 128
    n_groups_pad = n_groups + 2 * GPC

    nc = bacc.Bacc(None, target_bir_lowering=False, debug=False)

    emb_d = nc.dram_tensor("emb_table", [vocab, E], BF16, kind="ExternalInput")
    sw_d = nc.dram_tensor("s_w", [XROWS, 112], BF16, kind="ExternalInput")
    wxh_d = nc.dram_tensor("w_xh", [E, H], BF16, kind="ExternalInput")
    b0h_d = nc.dram_tensor("b0h", [H, 1], F32, kind="ExternalInput")
    offs_d = nc.dram_tensor("offs", [128, n_groups_pad], I32, kind="ExternalInput")
    out_d = nc.dram_tensor("h_final", [H, B], F32, kind="ExternalOutput")

    with tile.TileContext(nc) as tc:
        with ExitStack() as ctx:
            sb = ctx.enter_context(tc.tile_pool(name="sb", bufs=1))
            ps = ctx.enter_context(tc.tile_pool(name="ps", bufs=1, space="PSUM"))

            S_w = sb.tile([XROWS, 112], BF16)
            wxh96 = sb.tile([96, H], BF16)      # rows 64:96 = K_h
            hI48 = sb.tile([48, H], BF16)       # [I;0;I]
            b0h = sb.tile([H, 1], F32)
            ident = sb.tile([128, 128], BF16)
            offs = sb.tile([128, n_groups_pad], I32)
            XA = sb.tile([XROWS, Tc * B], BF16)
            XB = sb.tile([XROWS, Tc * B], BF16)
            stgA = sb.tile([128, GPC * E], BF16)
            stgB = sb.tile([128, GPC * E], BF16)
            owinA = sb.tile([128, GPC], I32)
            owinB = sb.tile([128, GPC], I32)
            xh_sb = [sb.tile([H, 512], F32, name=f"xhsb_{i}") for i in range(2)]
            sout = [sb.tile([80, Bg], F32, name=f"sout_{g}") for g in range(G)]
            q_sb = [sb.tile([H, Bg], F32, name=f"q_{g}") for g in range(G)]
            u_sb = [sb.tile([H, Bg], F32, name=f"u_{g}") for g in range(G)]
            ofin = sb.tile([H, B], F32)

            # full-bank PSUM tiles (bank-exclusive by construction)
            P = [ps.tile([128, 512], F32, name=f"P_{g}") for g in range(G)]
            H2p = [ps.tile([128, 512], F32, name=f"H2p_{g}") for g in range(G)]
            XH = [ps.tile([128, 512], F32, name=f"XH_{i}") for i in range(2)]
            TP = [ps.tile([128, 1024], BF16, name=f"TP_{i}") for i in range(2)]

            nc.sync.dma_start(out=S_w[:], in_=sw_d[:])
            nc.sync.dma_start(out=wxh96[64:96, :], in_=wxh_d[:])
            nc.sync.dma_start(out=b0h[:], in_=b0h_d[:])
            nc.sync.dma_start(out=offs[:], in_=offs_d[:])
            make_identity(nc, ident[:])
            nc.vector.memset(hI48[:], 0.0)
            nc.vector.memset(wxh96[0:64, :], 0.0)
            nc.vector.memset(XA[:], 0.0)
            nc.vector.memset(XB[:], 0.0)
            nc.vector.memset(XA[96:97, :], 1.0)
            nc.vector.memset(XB[96:97, :], 1.0)
            for g in range(G):
                nc.vector.memset(sout[g][:], 0.0)
                nc.vector.memset(q_sb[g][:], 0.0)
                nc.vector.memset(u_sb[g][:], 0.0)
            for i in range(2):
                nc.vector.memset(xh_sb[i][:], 0.0)
            nc.vector.memset(ofin[:], 0.0)
            # hI48 = [I;0;I] via iota compare would be complex; use identity copy
            nc.vector.tensor_copy(hI48[0:16, :], ident[0:16, 0:16])
            nc.vector.tensor_copy(hI48[32:48, :], ident[0:16, 0:16])

            def emit_owin(chunk, owin):
                if isinstance(chunk, int):
                    src = offs[:, chunk * GPC:(chunk + 1) * GPC]
                else:
                    src = offs[:, bass.ts(chunk, GPC)]
                nc.vector.tensor_copy(owin[:], src)

            def emit_gather_one(stg, owin, g):
                if NOGATHER:
                    return
                nc.gpsimd.indirect_dma_start(
                    out=stg[:, g * E:(g + 1) * E],
                    out_offset=None,
                    in_=emb_d[:],
                    in_offset=bass.IndirectOffsetOnAxis(ap=owin[:, g:g + 1], axis=0),
                )

            def emit_gather(chunk, stg, owin):
                emit_owin(chunk, owin)
                for g in range(GPC):
                    emit_gather_one(stg, owin, g)

            def prep_ops(stg, X):
                """Transpose gathered embeddings into X rows 64:96."""
                for blk in range(NBLK):
                    def tp_blk(blk=blk):
                        tp = TP[blk % 2]
                        for j in range(4):
                            gg = blk * 4 + j
                            nc.tensor.transpose(
                                out=tp[0:E, j * 128:(j + 1) * 128],
                                in_=stg[:, gg * E:(gg + 1) * E],
                                identity=ident[:],
                            )
                        nc.vector.tensor_copy(
                            X[64:96, blk * 512:(blk + 1) * 512], tp[0:E, 0:512])
                    yield tp_blk

            def emit_xh(X, blk, xbuf):
                """xh for steps [blk*16,(blk+1)*16) of X -> XH[xbuf] -> xh_sb."""
                nc.tensor.matmul(
                    XH[xbuf][0:H, :], wxh96[64:96, :],
                    X[64:96, blk * 512:(blk + 1) * 512],
                    start=True, stop=True, tile_position=(64, 0))
                nc.vector.tensor_copy(xh_sb[xbuf][:], XH[xbuf][0:H, :])

            def emit_step(X, Xn, t, tn):
                """One timestep, both groups, stage-major."""
                cols = [t * B + g * Bg for g in range(G)]
                ncols = [tn * B + g * Bg for g in range(G)]
                us = (t % UBLK) * B
                xbuf = (t // UBLK) % 2
                for g in range(G):
                    nc.tensor.matmul(H2p[g][0:H, 0:Bg], hI48[:],
                                     X[0:48, cols[g]:cols[g] + Bg],
                                     start=True, stop=True)
                    nc.tensor.matmul(P[g][0:112, 0:Bg], S_w[:],
                                     X[0:XROWS, cols[g]:cols[g] + Bg],
                                     start=True, stop=True)
                for g in range(G):
                    nc.scalar.activation(sout[g][:], P[g][0:80, 0:Bg], SIG)
                for g in range(G):
                    nc.vector.tensor_tensor(q_sb[g][:], P[g][96:112, 0:Bg],
                                            sout[g][64:80, :], op=MUL)
                for g in range(G):
                    ueng = nc.gpsimd if UENG == "gps" else nc.vector
                    ueng.tensor_tensor(
                        u_sb[g][:], q_sb[g][:],
                        xh_sb[xbuf][:, us + g * Bg:us + g * Bg + Bg], op=ADD)
                for g in range(G):
                    nc.scalar.activation(H2p[g][32:48, 0:Bg], u_sb[g][:], SIG,
                                         bias=b0h[:])
                for g in range(G):
                    nc.vector.tensor_tensor(Xn[0:48, ncols[g]:ncols[g] + Bg],
                                            sout[g][0:48, :],
                                            H2p[g][0:48, 0:Bg], op=MUL)

            def emit_chunk(X, Xn, next_chunk, stg, owin, preps):
                """Run Tc steps on X; interleave gather+prep of next chunk."""
                sched = {}
                # gather g of next chunk at step 4g (Tc/4 gathers per chunk);
                # transpose-block blk (needs gathers 4blk..4blk+3) at
                # step 16*blk + 14.
                sched.setdefault(0, []).append(lambda: emit_owin(next_chunk, owin))
                for g in range(GPC):
                    sched.setdefault((g * Tc) // GPC, []).append(
                        lambda g=g: emit_gather_one(stg, owin, g))
                for i, p in enumerate(preps):
                    sched.setdefault((i * Tc) // NBLK + 14, []).append(p)
                for t in range(Tc):
                    if t % UBLK == 0:
                        # xh for block t//UBLK + 1 (next block); last block
                        # preps block 0 of the NEXT chunk.
                        nb = t // UBLK + 1
                        if nb < NBLK:
                            emit_xh(X, nb, nb % 2)
                        else:
                            emit_xh(Xn, 0, 0)
                    tn = t + 1 if t + 1 < Tc else 0
                    emit_step(X, X if t + 1 < Tc else Xn, t, tn)
                    for p in sched.get(t, ()):
                        p()

            # --- prologue: gather+prep chunk 0 into A, xh block 0 ---
            emit_gather(0, stgA, owinA)
            for p in prep_ops(stgA, XA):
                p()
            emit_xh(XA, 0, 0)

            def body(i):
                emit_chunk(XA, XB, 2 * i + 1, stgB, owinB,
                           list(prep_ops(stgB, XB)))
                emit_chunk(XB, XA, 2 * i + 2, stgA, owinA,
                           list(prep_ops(stgA, XA)))

            if use_for_i:
                with tc.For_i(0, NBODY, 1,
                              hint_engines=(PEE, DVEE, ACTE)) as i:
                    body(i)
            else:
                for i in range(NBODY):
                    body(i)

            # final h = a + p2 from XA col 0 (state entering step T)
            nc.tensor.matmul(H2p[0][0:H, 0:B], hI48[:], XA[0:48, 0:B],
                             start=True, stop=True)
            nc.scalar.copy(out=ofin[:], in_=H2p[0][0:H, 0:B])
            nc.sync.dma_start(out=out_d[:], in_=ofin[:])

    nc.compile()
    return nc


def pack_inputs(ids_core, emb_table, kernel, rec_kernel, bias, T, Tc):
    """Host-side packing for one core. ids_core [32, T] int."""
    GPC = Tc * B // 128
    n_groups = T * B // 128
    n_groups_pad = n_groups + 2 * GPC
    R = np.asarray(rec_kernel, np.float32)          # [16, 48] cols z|r|h
    K = np.asarray(kernel, np.float32)              # [32, 48]
    b0, b1 = np.asarray(bias, np.float32)           # [48] each

    S = np.zeros((XROWS, 112), np.float32)
    # M cols: z 0:16 | pad | zn 32:48 | pad | r 64:80 | pad | rh 96:112
    for rows in (slice(0, 16), slice(32, 48)):      # a-rows, p2-rows
        S[rows, 0:16] = R[:, 0:16]
        S[rows, 32:48] = -R[:, 0:16]
        S[rows, 64:80] = R[:, 16:32]
        S[rows, 96:112] = R[:, 32:48]
    S[64:96, 0:16] = K[:, 0:16]
    S[64:96, 32:48] = -K[:, 0:16]
    S[64:96, 64:80] = K[:, 16:32]
    S[96, 0:16] = b0[0:16] + b1[0:16]
    S[96, 32:48] = -(b0[0:16] + b1[0:16])
    S[96, 64:80] = b0[16:32] + b1[16:32]
    S[96, 96:112] = b1[32:48]

    w_xh = np.asarray(K[:, 32:48], np.float32)      # [32, 16]
    b0h = b0[32:48].reshape(H, 1).astype(np.float32)

    flat = np.ascontiguousarray(ids_core.T).reshape(-1)   # i = t*32 + b
    offs = np.zeros((128, n_groups_pad), np.int32)
    offs[:, :n_groups] = flat.reshape(n_groups, 128).T.astype(np.int32)

    import ml_dtypes
    bf = lambda a: np.asarray(a, dtype=ml_dtypes.bfloat16)
    return {
        "emb_table": bf(emb_table),
        "s_w": bf(S),
        "w_xh": bf(w_xh),
        "b0h": b0h,
        "offs": offs,
    }


_NC_CACHE = {}


def _get_nc():
    key = (T, TC)
    if key not in _NC_CACHE:
        _NC_CACHE[key] = build_kernel(T=T, Tc=TC, vocab=VOCAB, use_for_i=True)
    return _NC_CACHE[key]


def make_in_maps(ids, emb_table, kern, rec_kernel, bias):
    ids = np.asarray(ids)
    assert ids.shape == (NCORES * B, T), ids.shape
    ids = ids.astype(np.int32, copy=False)
    return [
        pack_inputs(ids[c * B:(c + 1) * B], emb_table, kern, rec_kernel, bias,
                    T, TC)
        for c in range(NCORES)
    ]


def kernel(ids, emb_table, kernel, rec_kernel, bias):
    """Full inputs in, full output out. Shards batch 8 ways internally."""
    out_dtype = np.asarray(emb_table).dtype
    in_maps = make_in_maps(ids, emb_table, kernel, rec_kernel, bias)
    nc = _get_nc()
    res = run_bass_kernel_spmd(nc, in_maps, core_ids=list(range(NCORES)))
    out = np.concatenate(
        [res.results[c]["h_final"].T for c in range(NCORES)], axis=0
    ).astype(out_dtype, copy=False)
    return out
